# revision 77
# baseline (speedup 1.0000x reference)
"""GCN regressor (3x GCNConv + BatchNorm + ReLU) on 8 Trainium2 NeuronCores.

Sharding (graph/data parallel):
  - Nodes split into 8 blocks of 6250; a core owns the dsts in its block and
    all edges pointing at them. Rank r in a block -> (lane p=r%128, slot
    t=r//128), padded to 6272 rows (49 slots).
  - Per layer the dinv-prescaled feature table is exchanged in TWO
    AllGather waves (wave A = lanes 0..63 of every slot, wave B = lanes
    64..127). Each wave's compact [25088, 64] f16 output is restrided into a
    paired table [25088, 256B] (wave A in the low 128B of each row, B in the
    high 128B), so SWDGE dma_gather can address rows with the 256B-granular
    elem_step while int16 indices stay < 25088. Wave A's restride + gathers
    + reductions run concurrently with wave B's AllGather.
  - Sparse aggregation: gathered source rows land in per-stream window
    tiles; a DVE in-place binary-fold tree sums each dst slot's columns in
    f16 (messages are pre-scaled by dinv[src]; |sums| are O(1), f16 is
    plenty against the 2e-2 gate). Wave-A partials park in an f16
    accumulator; wave B folds the self-loop term + A partial, and the ACT
    engine drains with the dinv[dst] output scale. This replaces the
    baseline's per-column identity-matmul accumulation, which bottlenecked
    on PE sequencer dispatch (~185 ns per 128-edge matmul).
  - Nodes are assigned to waves by id parity; slot grouping packs 64 even-id
    + 64 odd-id dsts per slot, window-sorted by per-stream in-degree to
    minimize gather padding.
  - Dense math (x@W, BN, ReLU) on PE/DVE/ACT; BN statistics AllGathered and
    summed locally (biased variance, as the reference). b1/b2 are absorbed
    by BatchNorm.
  - The LAST layer exploits that Ahat commutes with per-node linear maps:
    it exchanges/aggregates scalars w = (dinv*h2)@W3 instead of 64-wide
    rows, shrinking its AllGather to 100KB (one merged collective) and its
    gather descriptors to the 7ns DMA floor; out = dinv*agg(w) + b3.
  - Layer 1's wave buffers (x*dinv) are precomputed on the host so the
    first AllGather launches immediately at kernel start.

kernel(**inputs) takes FULL inputs, returns the FULL [50000] output (f32).
"""

import sys

sys.path.insert(0, '/opt/trn_rl_repo')

import numpy as np

import concourse.bass as bass
import concourse.bacc as bacc
import concourse.tile as tile
import concourse.mybir as mybir
from concourse import library_config
from concourse.masks import make_identity


def _patch_dma_gather():
    """Allow 128-byte gather payloads (elem_step stays 256B-granular, which
    the InstDMAGatherAnt stride_bytes_256 ISA field requires)."""
    import inspect, textwrap
    src = textwrap.dedent(inspect.getsource(bass.BassGpSimd.dma_gather))
    src = src.replace("""    assert (
        elem_size_bytes > 0 and elem_size_bytes % 256 == 0
    )  # transpose restriction""", "    assert elem_size_bytes > 0")
    g = dict(bass.BassGpSimd.dma_gather.__globals__)
    exec(src, g)
    bass.BassGpSimd.dma_gather = g["dma_gather"]


_patch_dma_gather()

F32 = mybir.dt.float32
F16 = mybir.dt.float16
I16 = mybir.dt.int16
AF = mybir.ActivationFunctionType

D = 64
NC = 8
EPS = 1e-5
WSORT = 512         # window size for the per-class slot-grouping heuristic
MAXC = 28           # gather columns per dma_gather instruction
WCAP = 76           # stream window capacity in columns (slot-aligned)
WCAP3 = 192         # last-layer (scalar) window capacity


class Cfg:
    def __init__(self, n_nodes, n_cores=NC):
        self.n = n_nodes
        self.nc = n_cores
        self.nloc = n_nodes // n_cores
        assert self.nloc * n_cores == n_nodes
        self.slots = self.nloc // 128 + 1          # ensures pad ranks exist
        self.npad = self.slots * 128
        assert self.nloc < self.npad
        self.half = self.npad // 2                 # rows per wave per core
        self.prows = self.half * n_cores           # paired-table rows


class Sched:
    """Gather schedule: per-slot per-stream column counts, slot-aligned
    windows, fixed-size gather chunks, packed idx layout."""
    def __init__(self, ca, cb, windows, windows3, icols):
        self.ca = ca            # [S] stream-A columns per slot
        self.cb = cb            # [S] stream-B columns per slot
        self.windows = windows  # per stream: list of (t0, t1, cols, chunks)
                                # chunk = (icol0, m, col_off_in_window)
        self.windows3 = windows3   # big-window variant for the scalar layer
        self.icols = icols
        self.key = (tuple(ca), tuple(cb),
                    tuple((s, t0, t1, c, tuple(ch)) for s, ws in
                          enumerate(windows + windows3)
                          for (t0, t1, c, ch) in ws))


def host_prep(cfg, edge_index):
    n, nc_, nloc, S = cfg.n, cfg.nc, cfg.nloc, cfg.slots
    src = np.asarray(edge_index[0], dtype=np.int64)
    dst = np.asarray(edge_index[1], dtype=np.int64)
    deg = np.bincount(dst, minlength=n).astype(np.int64) + 1   # + self loop

    cls = (src & 1).astype(np.int64)           # stream class = src id parity
    cntA = np.bincount(dst[cls == 0], minlength=n)
    cntB = np.bincount(dst[cls == 1], minlength=n)

    # Placement: per core, even-id nodes on lanes 0..63, odd-id on 64..127.
    # Slot grouping: window-sort each class list by (cntA desc, then cntB
    # desc within windows); slot t takes element t of each 64-wide stripe.
    perms = []            # rank r -> node id (-1 = pad), r = t*128 + p
    for c in range(nc_):
        nodes = np.arange(c * nloc, (c + 1) * nloc)
        halves = []
        for parity in (0, 1):
            cand = nodes[nodes % 2 == parity]
            o = np.argsort(-cntA[cand], kind="stable")
            parts = []
            for s0 in range(0, len(o), WSORT):
                blk = o[s0:s0 + WSORT]
                parts.append(blk[np.argsort(-cntB[cand[blk]], kind="stable")])
            ordered = cand[np.concatenate(parts)] if parts else cand
            full = np.full(cfg.half, -1, dtype=np.int64)
            full[:len(ordered)] = ordered
            halves.append(full)
        perm = np.full(cfg.npad, -1, dtype=np.int64)
        for t in range(S):
            perm[t * 128:t * 128 + 64] = halves[0][t * 64:(t + 1) * 64]
            perm[t * 128 + 64:(t + 1) * 128] = halves[1][t * 64:(t + 1) * 64]
        perms.append(perm)

    # node -> (rank, core); pair index of node m: core*half + t*64 + (p%64)
    rank = np.zeros(n, dtype=np.int64)
    for c in range(nc_):
        m = perms[c] >= 0
        rank[perms[c][m]] = np.nonzero(m)[0]
    t_of = rank // 128
    p_of = rank % 128
    # q-major pair index: row = (p%64)*S + t, so each publish partition is
    # one contiguous 6272B DMA descriptor
    pair_of = (np.int64(cfg.half) * (np.arange(n) // nloc)
               + (p_of % 64) * S + t_of)
    zpair = cfg.half - 1       # block-relative filler pair (pad cell)

    # per-core per-lane streams; global per-slot column counts
    ca = np.zeros(S, dtype=np.int64)
    cb = np.zeros(S, dtype=np.int64)
    lane_lists = []
    for c in range(nc_):
        m = (dst // nloc) == c
        s_c, d_c = src[m], dst[m]
        r_c = rank[d_c]
        h_c = cls[np.arange(len(src))[m]]
        order = np.lexsort((pair_of[s_c], h_c, r_c))
        s_c, r_c, h_c = s_c[order], r_c[order], h_c[order]
        rows = pair_of[s_c]
        cnt0 = np.bincount(r_c[h_c == 0], minlength=cfg.npad)
        cnt1 = np.bincount(r_c[h_c == 1], minlength=cfg.npad)
        for t in range(S):
            ca[t] = max(ca[t], cnt0[t * 128:(t + 1) * 128].max())
            cb[t] = max(cb[t], cnt1[t * 128:(t + 1) * 128].max())
        lane_lists.append((rows[h_c == 0], r_c[h_c == 0], cnt0,
                           rows[h_c == 1], r_c[h_c == 1], cnt1))

    for t in range(S):
        assert ca[t] <= WCAP and cb[t] <= WCAP, (t, ca[t], cb[t])

    # slot-aligned windows per stream; idx columns are packed linearly in
    # stream order (A then B), so any chunking slices the same idx layout
    sbases = (0, int(ca.sum()))
    icol = int((ca.sum() + cb.sum()) * 8)

    def build_windows(cap, chunkcap, lead):
        res = ([], [])
        for sidx, cc in ((0, ca), (1, cb)):
            t = 0
            spos = 0
            while t < S:
                t0, cols = t, 0
                while t < S and cols + cc[t] <= cap and not (
                        t == S - 1 and t > t0):
                    cols += cc[t]
                    t += 1
                chunks = []
                done = 0
                while done < cols:
                    # small first chunk per stream: its descriptor
                    # generation is the DMA lead-in after the restride
                    mcols = min(lead if spos + done == 0 else chunkcap,
                                cols - done)
                    chunks.append(((sbases[sidx] + spos + done) * 8,
                                   int(mcols), int(done)))
                    done += mcols
                res[sidx].append((t0, t, int(cols), tuple(chunks)))
                spos += cols
        return (tuple(res[0]), tuple(res[1]))

    windows = build_windows(WCAP, MAXC, 24)

    windows3 = build_windows(WCAP3, 96, 48)

    # idx tensors: stream A flat columns then stream B, in slot order,
    # wrapped into 16 partitions and replicated x8 (SWDGE idx layout).
    idxs = np.zeros((nc_, 128, max(icol, 1)), dtype=np.int16)
    for c in range(nc_):
        rowsA, rA, cnt0, rowsB, rB, cnt1 = lane_lists[c]
        startsA = np.concatenate([[0], np.cumsum(cnt0)])
        startsB = np.concatenate([[0], np.cumsum(cnt1)])
        zfill = c * cfg.half + zpair
        flat_cols = []
        for cc, rows_, starts, cnt in ((ca, rowsA, startsA, cnt0),
                                       (cb, rowsB, startsB, cnt1)):
            for t in range(S):
                ranks = t * 128 + np.arange(128)
                for k in range(int(cc[t])):
                    col = np.full(128, zfill, dtype=np.int64)
                    have = cnt[ranks] > k
                    col[have] = rows_[starts[ranks[have]] + k]
                    flat_cols.append(col.astype(np.int16))
        if flat_cols:
            fc = np.stack(flat_cols)
            flat = fc.reshape(-1)
            wrapped = np.zeros((16, icol), np.int16)
            kk = np.arange(len(flat))
            wrapped[kk % 16, kk // 16] = flat
            idxs[c] = np.tile(wrapped, (8, 1))
    sched = Sched([int(x) for x in ca], [int(x) for x in cb],
                  windows, windows3, max(icol, 1))
    return deg, perms, sched, idxs


def build(cfg, sched):
    nc = bacc.Bacc("TRN2", target_bir_lowering=False, debug=False,
                   enable_asserts=False, num_devices=cfg.nc,
                   num_swdge_queues=4)
    S = cfg.slots
    NPF = S * 64
    NP = cfg.npad
    PR = cfg.prows
    HF = cfg.half
    ca, cb = sched.ca, sched.cb

    xl = nc.dram_tensor("xl", [128, NPF], F16, kind="ExternalInput").ap()
    xa = nc.dram_tensor("xa", [HF, D], F16, kind="ExternalInput").ap()
    xb = nc.dram_tensor("xb", [HF, D], F16, kind="ExternalInput").ap()
    degt = nc.dram_tensor("degt", [128, S], F32, kind="ExternalInput").ap()
    idx = nc.dram_tensor("idx", [128, sched.icols], I16,
                         kind="ExternalInput").ap()
    w1 = nc.dram_tensor("w1", [D, D], F32, kind="ExternalInput").ap()
    w2 = nc.dram_tensor("w2", [D, D], F32, kind="ExternalInput").ap()
    w3 = nc.dram_tensor("w3", [D, 1], F32, kind="ExternalInput").ap()
    gb = nc.dram_tensor("gb", [4, D], F32, kind="ExternalInput").ap()
    b3r = nc.dram_tensor("b3r", [128, 1], F32, kind="ExternalInput").ap()
    out = nc.dram_tensor("out", [1, NP], F32, kind="ExternalOutput").ap()

    inv_n = 1.0 / cfg.n

    with tile.TileContext(nc) as tc:
        with tc.tile_pool(name="const", bufs=1) as cpool, \
             tc.tile_pool(name="stga", bufs=3) as spool_a, \
             tc.tile_pool(name="stgb", bufs=3) as spool_b, \
             tc.tile_pool(name="ps", bufs=2, space="PSUM") as ppool, \
             tc.tile_pool(name="ps1", bufs=1, space="PSUM") as ppool1, \
             tc.tile_pool(name="dram", bufs=1, space="DRAM") as dpool:

            nc.gpsimd.load_library(library_config.mlp)

            # layer-1 AllGather inputs first in the DMA queue so AG_A can
            # launch ~15us earlier (everything else loads during it)
            agA = dpool.tile([HF, D], F16, name="agA")
            agB = dpool.tile([HF, D], F16, name="agB")
            nc.sync.dma_start(agA[:], xa[:])
            nc.sync.dma_start(agB[:], xb[:])

            ident = cpool.tile([128, 128], F32)
            make_identity(nc, ident[:])
            w1t = cpool.tile([D, D], F32)
            w2t = cpool.tile([D, D], F32)
            w3t = cpool.tile([D, 1], F32)
            gbt = cpool.tile([4, D], F32)
            b3rt = cpool.tile([128, 1], F32)
            epst = cpool.tile([D, 1], F32)
            nc.sync.dma_start(w1t[:], w1[:])
            nc.sync.dma_start(w2t[:], w2[:])
            nc.sync.dma_start(w3t[:], w3[:])
            nc.sync.dma_start(gbt[:], gb[:])
            nc.sync.dma_start(b3rt[:], b3r[:])
            nc.vector.memset(epst[:], EPS)
            idxt = cpool.tile([128, sched.icols], I16)
            nc.sync.dma_start(idxt[:], idx[:])
            degs = cpool.tile([128, S], F32)
            nc.sync.dma_start(degs[:], degt[:])

            # gbT [64, 4] = (g1, bt1, g2, bt2) columns
            pgb = ppool1.tile([D, 4], F32, name="pgb")
            nc.tensor.transpose(pgb[:], gbt[:], ident[:4, :4])
            gbs = cpool.tile([D, 4], F32)
            nc.vector.tensor_copy(out=gbs[:], in_=pgb[:])

            dinv = cpool.tile([128, S], F32)
            nc.scalar.sqrt(dinv[:], degs[:])
            nc.vector.reciprocal(dinv[:], dinv[:])
            hcast = cpool.tile([128, NPF], F16)
            wcast = cpool.tile([128, S], F16, name="wcast")
            wrow = cpool.tile([1, NP], F32, name="wrow")
            wq = cpool.tile([128, S], F32, name="wq")
            osum = cpool.tile([128, S], F32, name="osum")
            accA = cpool.tile([128, NPF], F16)
            acc_f32 = cpool.tile([128, NPF], F32, name="accf")
            nc.sync.dma_start(hcast[:], xl[:])      # host pre-scaled x*dinv
            dinv_exp = cpool.tile([128, NPF], F32, name="dinv_exp")
            for t in range(S):
                nc.vector.tensor_copy(
                    out=dinv_exp[:, t * 64:(t + 1) * 64],
                    in_=dinv[:, t:t + 1].to_broadcast([128, 64]))

            # wave publish buffers + AllGather outputs (pair-HBM shared)
            agC3 = dpool.tile([2 * HF, 1], F16, name="agC3")
            tabA = nc.dram_tensor("tabA", [PR, D], F16,
                                  addr_space="Shared").ap()
            tabB = nc.dram_tensor("tabB", [PR, D], F16,
                                  addr_space="Shared").ap()
            tabC3 = nc.dram_tensor("tabC3", [2 * PR, 1], F16,
                                   addr_space="Shared").ap()
            # paired gather table: row r = (wave-A node r, wave-B node r)
            tabs = [dpool.tile([PR, 2 * D], F16, name=f"tab{l}")
                    for l in range(3)]
            wrowd = dpool.tile([1, NP], F32, name="wrowd")
            ar_in = dpool.tile([D, 2], F32, name="ar_in")
            ar_out = [dpool.tile([NC * D, 2], F32, name=f"ar_out{l}")
                      for l in range(2)]

            yT = cpool.tile([D, NP], F32, name="yT")
            zT = cpool.tile([D, NP], F32, name="zT")
            st = cpool.tile([D, 2], F32, name="st")
            sta = cpool.tile([D, 16], F32, name="sta")
            stb = cpool.tile([D, 16], F32, name="stb")
            stw = cpool.tile([D, 16], F32, name="stw")
            stg = cpool.tile([D, 2], F32, name="stg")
            scb = cpool.tile([D, 4], F32, name="scb")
            msq = cpool.tile([D, 1], F32, name="msq")
            rstd = cpool.tile([D, 1], F32, name="rstd")

            nb = (NP + 511) // 512

            qi = 0
            for layer in range(3):
                # publish the two wave buffers from hcast (or, for the last
                # layer, scalar wcast = dinv*(h2@W3)) lane halves
                EW = 1 if layer == 2 else D     # table row payload elements
                srct = wcast if layer == 2 else hcast
                tab = tabs[layer]
                if layer < 2:
                    inA, inB = agA, agB
                    if layer != 0:    # layer 0 staged at kernel start
                        nc.sync.dma_start(
                            agA.rearrange("(q t) f -> q t f", t=S),
                            hcast[0:64, :].rearrange("q (t f) -> q t f", f=D))
                        nc.sync.dma_start(
                            agB.rearrange("(q t) f -> q t f", t=S),
                            hcast[64:128, :].rearrange("q (t f) -> q t f", f=D))
                    if cfg.nc > 1:
                        nc.gpsimd.collective_compute(
                            "AllGather", mybir.AluOpType.bypass,
                            replica_groups=[list(range(cfg.nc))],
                            ins=[inA.opt()], outs=[tabA.opt()],
                        )
                        nc.gpsimd.collective_compute(
                            "AllGather", mybir.AluOpType.bypass,
                            replica_groups=[list(range(cfg.nc))],
                            ins=[inB.opt()], outs=[tabB.opt()],
                        )
                    else:
                        nc.sync.dma_start(tabA[0:HF, :], inA[:])
                        nc.sync.dma_start(tabB[0:HF, :], inB[:])
                    # restride each wave into its half of the 256B paired
                    # rows; wave A's copy (and its gathers) overlap wave B's
                    # AllGather
                    nc.sync.dma_start(tab[:, 0:D], tabA[:])
                    nc.sync.dma_start(tab[:, D:2 * D], tabB[:])
                else:
                    # last layer: one tiny scalar AllGather for both waves
                    nc.sync.dma_start(
                        agC3[0:HF, :].rearrange("(q t) f -> q t f", t=S),
                        wcast[0:64, :].rearrange("q (t f) -> q t f", f=1))
                    nc.sync.dma_start(
                        agC3[HF:2 * HF, :].rearrange("(q t) f -> q t f", t=S),
                        wcast[64:128, :].rearrange("q (t f) -> q t f", f=1))
                    if cfg.nc > 1:
                        nc.gpsimd.collective_compute(
                            "AllGather", mybir.AluOpType.bypass,
                            replica_groups=[list(range(cfg.nc))],
                            ins=[agC3.opt()], outs=[tabC3.opt()],
                        )
                    else:
                        nc.sync.dma_start(tabC3[0:2 * HF, :], agC3[:])
                    for c in range(cfg.nc):
                        nc.sync.dma_start(
                            tab[c * HF:(c + 1) * HF, 0:1],
                            tabC3[c * 2 * HF:c * 2 * HF + HF, :])
                        nc.sync.dma_start(
                            tab[c * HF:(c + 1) * HF, D:D + 1],
                            tabC3[c * 2 * HF + HF:(c + 1) * 2 * HF, :])
                tviews = (tab[:, 0:EW], tab[:, D:D + EW])

                def emit_transpose(u):
                    pt = ppool.tile([D, 128], F32, tag="tp", name="pt")
                    nc.tensor.transpose(pt[:], acc_f32[:, u * 64:(u + 1) * 64],
                                        ident[:])
                    nc.vector.tensor_copy(out=yT[:, u * 128:(u + 1) * 128],
                                          in_=pt[:])

                def emit_zblock(b):
                    c0_, c1_ = b * 512, min(NP, b * 512 + 512)
                    pz = ppool.tile([D, 512], F32, tag="pz", name="pz")
                    nc.tensor.matmul(pz[:, :c1_ - c0_],
                                     (w1t, w2t)[layer][:],
                                     yT[:, c0_:c1_], start=True, stop=True)
                    nc.vector.tensor_copy(out=zT[:, c0_:c1_],
                                          in_=pz[:, :c1_ - c0_])
                    nc.scalar.activation(yT[:, c0_:c1_], zT[:, c0_:c1_],
                                         AF.Copy,
                                         accum_out=sta[:, b:b + 1])
                    nc.scalar.activation(yT[:, c0_:c1_], zT[:, c0_:c1_],
                                         AF.Square,
                                         accum_out=stb[:, b:b + 1])

                def fold(wt, base, k, ew):
                    """In-place binary-fold of k columns [128, k*ew] f16 at
                    column `base` of window tile wt; result lands at base."""
                    while k > 1:
                        h = k // 2      # top h columns fold onto the first h
                        nc.vector.tensor_add(
                            out=wt[:, base * ew:(base + h) * ew],
                            in0=wt[:, base * ew:(base + h) * ew],
                            in1=wt[:, (base + k - h) * ew:(base + k) * ew])
                        k -= h

                # ---- wave A: gathers + per-slot partial sums into accA;
                # ---- wave B: gathers + combine + drain ----
                wins = sched.windows3 if layer == 2 else sched.windows
                wcap_l = WCAP3 if layer == 2 else WCAP
                for sidx, cs, pool, target in (
                        (0, ca, spool_a, "A"), (1, cb, spool_b, "B")):
                    for (t0, t1, cols, chunks) in wins[sidx]:
                        wt = pool.tile([128, wcap_l * EW], F16,
                                       name=f"w{target}{EW}")
                        for (icol0, m, coff) in chunks:
                            nc.gpsimd.dma_gather(
                                wt[:, coff * EW:(coff + m) * EW]
                                .rearrange("p (m x) -> p m x", x=EW),
                                tviews[sidx], idxt[:, icol0:icol0 + 8 * m],
                                128 * m, 128 * m, EW, elem_step=2 * D,
                                single_packet=False, queue_num=qi % 4)
                            qi += 1
                        off = 0
                        for t in range(t0, t1):
                            k = cs[t]
                            if sidx == 0:
                                # fold slot; stash partial + self-loop term
                                if k > 0:
                                    fold(wt, off, k, EW)
                                    nc.vector.tensor_add(
                                        out=accA[:, t * EW:(t + 1) * EW],
                                        in0=wt[:, off * EW:(off + 1) * EW],
                                        in1=srct[:, t * EW:(t + 1) * EW])
                                else:
                                    nc.vector.tensor_copy(
                                        out=accA[:, t * EW:(t + 1) * EW],
                                        in_=srct[:, t * EW:(t + 1) * EW])
                            else:
                                # fold slot + A partial (has self-loop), drain
                                if k > 0:
                                    fold(wt, off, k, EW)
                                    nc.vector.tensor_add(
                                        out=wt[:, off * EW:(off + 1) * EW],
                                        in0=wt[:, off * EW:(off + 1) * EW],
                                        in1=accA[:, t * EW:(t + 1) * EW])
                                    srcap = wt[:, off * EW:(off + 1) * EW]
                                else:
                                    srcap = accA[:, t * EW:(t + 1) * EW]
                                if layer == 2:
                                    # out = dinv*agg, collected per slot
                                    nc.scalar.activation(
                                        osum[:, t:t + 1], srcap, AF.Copy,
                                        scale=dinv[:, t:t + 1])
                                    if t == 24:
                                        # first-half output while the rest
                                        # still gathers
                                        nc.vector.tensor_add(
                                            out=osum[:, 0:25],
                                            in0=osum[:, 0:25],
                                            in1=b3rt[:, 0:1]
                                            .to_broadcast([128, 25]))
                                        nc.sync.dma_start(
                                            out.rearrange(
                                                "a (t p) -> p (a t)",
                                                p=128)[:, 0:25],
                                            osum[:, 0:25])
                                else:
                                    nc.scalar.activation(
                                        acc_f32[:, t * 64:(t + 1) * 64],
                                        srcap, AF.Copy,
                                        scale=dinv[:, t:t + 1])
                                    if t >= 1:
                                        emit_transpose(t - 1)
                                        if (t - 1) % 4 == 3:
                                            emit_zblock((t - 1) // 4)
                            off += k

                if layer == 2:
                    nc.vector.tensor_add(
                        out=osum[:, 25:S], in0=osum[:, 25:S],
                        in1=b3rt[:, 0:1].to_broadcast([128, S - 25]))
                    nc.sync.dma_start(
                        out.rearrange("a (t p) -> p (a t)", p=128)[:, 25:S],
                        osum[:, 25:S])
                else:
                    emit_transpose(S - 1)
                    for b in range((S - 2) // 4 + 1, nb):
                        emit_zblock(b)

                if layer < 2:
                    nc.scalar.activation(stw[:, 0:nb], sta[:, 0:nb],
                                         AF.Copy, scale=inv_n,
                                         accum_out=st[:, 0:1])
                    nc.scalar.activation(stw[:, 0:nb], stb[:, 0:nb],
                                         AF.Copy, scale=inv_n,
                                         accum_out=st[:, 1:2])
                    nc.sync.dma_start(ar_in[:], st[:])
                    if cfg.nc > 1:
                        nc.gpsimd.collective_compute(
                            "AllGather", mybir.AluOpType.bypass,
                            replica_groups=[list(range(cfg.nc))],
                            ins=[ar_in.opt()], outs=[ar_out[layer].opt()],
                        )
                        nc.sync.dma_start(
                            stw[:].rearrange("d (c s) -> d c s", s=2),
                            ar_out[layer].rearrange("(c d) s -> d c s", d=D))
                        nc.vector.tensor_add(out=stw[:, 0:8], in0=stw[:, 0:8],
                                             in1=stw[:, 8:16])
                        nc.vector.tensor_add(out=stw[:, 0:4], in0=stw[:, 0:4],
                                             in1=stw[:, 4:8])
                        nc.vector.tensor_add(out=stg[:], in0=stw[:, 0:2],
                                             in1=stw[:, 2:4])
                    else:
                        nc.sync.dma_start(ar_out[layer][0:D, :], ar_in[:])
                        nc.sync.dma_start(stg[:], ar_out[layer][0:D, :])
                    nc.vector.tensor_copy(out=scb[:, 0:2], in_=stg[:, 0:2])
                    nc.vector.tensor_mul(out=msq[:], in0=scb[:, 0:1],
                                         in1=scb[:, 0:1])
                    nc.vector.tensor_sub(out=scb[:, 1:2], in0=scb[:, 1:2],
                                         in1=msq[:])
                    nc.scalar.activation(rstd[:], scb[:, 1:2], AF.Sqrt,
                                         bias=epst[:, 0:1])
                    nc.vector.reciprocal(rstd[:], rstd[:])
                    nc.vector.tensor_mul(out=scb[:, 2:3],
                                         in0=gbs[:, 2 * layer:2 * layer + 1],
                                         in1=rstd[:])
                    nc.vector.tensor_mul(out=msq[:], in0=scb[:, 0:1],
                                         in1=scb[:, 2:3])
                    nc.vector.tensor_sub(out=scb[:, 3:4],
                                         in0=gbs[:, 2 * layer + 1:2 * layer + 2],
                                         in1=msq[:])
                    # h.T = Relu(scale*z + bias); hcast = dinv * h
                    for b_ in range(nb):
                        c0_, c1_ = b_ * 512, min(NP, b_ * 512 + 512)
                        nc.scalar.activation(yT[:, c0_:c1_], zT[:, c0_:c1_],
                                             AF.Relu, bias=scb[:, 3:4],
                                             scale=scb[:, 2:3])
                        if layer == 1:
                            # w row = W3^T @ h2 on the idle PE (Ahat commutes
                            # with the per-node map W3)
                            pw = ppool.tile([1, 512], F32, tag="pw",
                                            name="pw")
                            nc.tensor.matmul(pw[:, :c1_ - c0_], w3t[:],
                                             yT[:, c0_:c1_], start=True,
                                             stop=True)
                            # split psum->wrow copies across DVE/ACT so
                            # neither queue paces the rebuild alone
                            if b_ % 3 != 0:
                                nc.vector.tensor_copy(
                                    out=wrow[0:1, c0_:c1_],
                                    in_=pw[:, :c1_ - c0_])
                            else:
                                nc.scalar.activation(wrow[0:1, c0_:c1_],
                                                     pw[:, :c1_ - c0_],
                                                     AF.Copy)
                        ns_ = (c1_ - c0_) // 128
                        ph4 = ppool.tile([128, 4 * D], F32, tag="tp",
                                         name="ph")
                        for j, t in enumerate(range(c0_ // 128, c1_ // 128)):
                            nc.tensor.transpose(
                                ph4[:, j * D:(j + 1) * D],
                                yT[:, t * 128:(t + 1) * 128],
                                ident[:64, :64])
                        nc.vector.tensor_mul(
                            out=hcast[:, c0_ // 2:c1_ // 2],
                            in0=ph4[:, 0:ns_ * D],
                            in1=dinv_exp[:, c0_ // 2:c1_ // 2])
                    if layer == 1:
                        # bounce w row through DRAM into node-major [128, S]
                        nc.sync.dma_start(wrowd[:], wrow[:])
                        nc.sync.dma_start(
                            wq[:],
                            wrowd.rearrange("a (t p) -> p (a t)", p=128))
                        nc.vector.tensor_mul(out=wcast[:], in0=wq[:],
                                             in1=dinv[:])
                    # pad rows self-zero: their dinv is ~1e-19 (deg=1e38)

    nc.compile()
    return nc


def make_in_maps(cfg, inputs, deg, perms, idxs):
    x = np.asarray(inputs["x"], dtype=np.float32)
    in_maps = []
    for c in range(cfg.nc):
        pm = perms[c]
        m = pm >= 0
        xp = np.zeros((cfg.npad, D), np.float32)
        xp[m] = x[pm[m]] / np.sqrt(deg[pm[m]].astype(np.float32))[:, None]
        xlc = xp.reshape(cfg.slots, 128, D).transpose(1, 0, 2) \
                .reshape(128, -1).astype(np.float16)
        dg = np.full((cfg.npad,), 1e30, np.float32)
        dg[m] = deg[pm[m]].astype(np.float32)
        dgt = dg.reshape(cfg.slots, 128).T.copy()
        # wave publish buffers for layer 1: row q*S+t = xlc[q (+64), t*64:]
        x3 = xlc.reshape(128, cfg.slots, D)
        xa_ = x3[0:64].reshape(cfg.half, D)
        xb_ = x3[64:128].reshape(cfg.half, D)
        in_maps.append({
            "xl": np.ascontiguousarray(xlc),
            "xa": np.ascontiguousarray(xa_),
            "xb": np.ascontiguousarray(xb_),
            "degt": np.ascontiguousarray(dgt),
            "idx": np.ascontiguousarray(idxs[c]),
            "w1": np.asarray(inputs["W1"], np.float32),
            "w2": np.asarray(inputs["W2"], np.float32),
            "w3": np.asarray(inputs["W3"], np.float32).reshape(D, 1),
            "gb": np.stack([
                np.asarray(inputs["g1"], np.float32),
                np.asarray(inputs["bt1"], np.float32),
                np.asarray(inputs["g2"], np.float32),
                np.asarray(inputs["bt2"], np.float32)]),
            "b3r": np.full((128, 1), float(np.asarray(inputs["b3"])
                                           .reshape(-1)[0]), np.float32),
        })
    return in_maps


_CACHE = {}


def kernel(**inputs):
    cfg = Cfg(n_nodes=int(np.asarray(inputs["x"]).shape[0]), n_cores=NC)
    deg, perms, sched, idxs = host_prep(
        cfg, np.asarray(inputs["edge_index"]))

    key = (cfg.n, sched.key)
    if key not in _CACHE:
        _CACHE[key] = build(cfg, sched)
    nc = _CACHE[key]
    in_maps = make_in_maps(cfg, inputs, deg, perms, idxs)

    import concourse.bass_utils as bass_utils
    res = None
    for attempt in range(3):
        try:
            res = bass_utils.run_bass_kernel_spmd(
                nc, in_maps, core_ids=list(range(cfg.nc)))
            break
        except Exception:
            if attempt == 2:
                raise
    out = np.zeros((cfg.n,), np.float32)
    for c in range(cfg.nc):
        oc = np.asarray(res.results[c]["out"]).reshape(cfg.npad)
        m = perms[c] >= 0
        out[perms[c][m]] = oc[m]
    return out


# revision 78
# speedup vs baseline: 1.0002x; 1.0002x over previous
"""GCN regressor (3x GCNConv + BatchNorm + ReLU) on 8 Trainium2 NeuronCores.

Sharding (graph/data parallel):
  - Nodes split into 8 blocks of 6250; a core owns the dsts in its block and
    all edges pointing at them. Rank r in a block -> (lane p=r%128, slot
    t=r//128), padded to 6272 rows (49 slots).
  - Per layer the dinv-prescaled feature table is exchanged in TWO
    AllGather waves (wave A = lanes 0..63 of every slot, wave B = lanes
    64..127). Each wave's compact [25088, 64] f16 output is restrided into a
    paired table [25088, 256B] (wave A in the low 128B of each row, B in the
    high 128B), so SWDGE dma_gather can address rows with the 256B-granular
    elem_step while int16 indices stay < 25088. Wave A's restride + gathers
    + reductions run concurrently with wave B's AllGather.
  - Sparse aggregation: gathered source rows land in per-stream window
    tiles; a DVE in-place binary-fold tree sums each dst slot's columns in
    f16 (messages are pre-scaled by dinv[src]; |sums| are O(1), f16 is
    plenty against the 2e-2 gate). Wave-A partials park in an f16
    accumulator; wave B folds the self-loop term + A partial, and the ACT
    engine drains with the dinv[dst] output scale. This replaces the
    baseline's per-column identity-matmul accumulation, which bottlenecked
    on PE sequencer dispatch (~185 ns per 128-edge matmul).
  - Nodes are assigned to waves by id parity; slot grouping packs 64 even-id
    + 64 odd-id dsts per slot, window-sorted by per-stream in-degree to
    minimize gather padding.
  - Dense math (x@W, BN, ReLU) on PE/DVE/ACT; BN statistics AllGathered and
    summed locally (biased variance, as the reference). b1/b2 are absorbed
    by BatchNorm.
  - The LAST layer exploits that Ahat commutes with per-node linear maps:
    it exchanges/aggregates scalars w = (dinv*h2)@W3 instead of 64-wide
    rows, shrinking its AllGather to 100KB (one merged collective) and its
    gather descriptors to the 7ns DMA floor; out = dinv*agg(w) + b3.
  - Layer 1's wave buffers (x*dinv) are precomputed on the host so the
    first AllGather launches immediately at kernel start.

kernel(**inputs) takes FULL inputs, returns the FULL [50000] output (f32).
"""

import sys

sys.path.insert(0, '/opt/trn_rl_repo')

import numpy as np

import concourse.bass as bass
import concourse.bacc as bacc
import concourse.tile as tile
import concourse.mybir as mybir
from concourse import library_config
from concourse.masks import make_identity


def _patch_dma_gather():
    """Allow 128-byte gather payloads (elem_step stays 256B-granular, which
    the InstDMAGatherAnt stride_bytes_256 ISA field requires)."""
    import inspect, textwrap
    src = textwrap.dedent(inspect.getsource(bass.BassGpSimd.dma_gather))
    src = src.replace("""    assert (
        elem_size_bytes > 0 and elem_size_bytes % 256 == 0
    )  # transpose restriction""", "    assert elem_size_bytes > 0")
    g = dict(bass.BassGpSimd.dma_gather.__globals__)
    exec(src, g)
    bass.BassGpSimd.dma_gather = g["dma_gather"]


_patch_dma_gather()

F32 = mybir.dt.float32
F16 = mybir.dt.float16
I16 = mybir.dt.int16
AF = mybir.ActivationFunctionType

D = 64
NC = 8
EPS = 1e-5
WSORT = 512         # window size for the per-class slot-grouping heuristic
MAXC = 28           # gather columns per dma_gather instruction
WCAP = 76           # stream window capacity in columns (slot-aligned)
WCAP3 = 192         # last-layer (scalar) window capacity


class Cfg:
    def __init__(self, n_nodes, n_cores=NC):
        self.n = n_nodes
        self.nc = n_cores
        self.nloc = n_nodes // n_cores
        assert self.nloc * n_cores == n_nodes
        self.slots = self.nloc // 128 + 1          # ensures pad ranks exist
        self.npad = self.slots * 128
        assert self.nloc < self.npad
        self.half = self.npad // 2                 # rows per wave per core
        self.prows = self.half * n_cores           # paired-table rows


class Sched:
    """Gather schedule: per-slot per-stream column counts, slot-aligned
    windows, fixed-size gather chunks, packed idx layout."""
    def __init__(self, ca, cb, windows, windows3, icols):
        self.ca = ca            # [S] stream-A columns per slot
        self.cb = cb            # [S] stream-B columns per slot
        self.windows = windows  # per stream: list of (t0, t1, cols, chunks)
                                # chunk = (icol0, m, col_off_in_window)
        self.windows3 = windows3   # big-window variant for the scalar layer
        self.icols = icols
        self.key = (tuple(ca), tuple(cb),
                    tuple((s, t0, t1, c, tuple(ch)) for s, ws in
                          enumerate(windows + windows3)
                          for (t0, t1, c, ch) in ws))


def host_prep(cfg, edge_index):
    n, nc_, nloc, S = cfg.n, cfg.nc, cfg.nloc, cfg.slots
    src = np.asarray(edge_index[0], dtype=np.int64)
    dst = np.asarray(edge_index[1], dtype=np.int64)
    deg = np.bincount(dst, minlength=n).astype(np.int64) + 1   # + self loop

    cls = (src & 1).astype(np.int64)           # stream class = src id parity
    cntA = np.bincount(dst[cls == 0], minlength=n)
    cntB = np.bincount(dst[cls == 1], minlength=n)

    # Placement: per core, even-id nodes on lanes 0..63, odd-id on 64..127.
    # Slot grouping: window-sort each class list by (cntA desc, then cntB
    # desc within windows); slot t takes element t of each 64-wide stripe.
    perms = []            # rank r -> node id (-1 = pad), r = t*128 + p
    for c in range(nc_):
        nodes = np.arange(c * nloc, (c + 1) * nloc)
        halves = []
        for parity in (0, 1):
            cand = nodes[nodes % 2 == parity]
            o = np.argsort(-cntA[cand], kind="stable")
            parts = []
            for s0 in range(0, len(o), WSORT):
                blk = o[s0:s0 + WSORT]
                parts.append(blk[np.argsort(-cntB[cand[blk]], kind="stable")])
            ordered = cand[np.concatenate(parts)] if parts else cand
            full = np.full(cfg.half, -1, dtype=np.int64)
            full[:len(ordered)] = ordered
            halves.append(full)
        perm = np.full(cfg.npad, -1, dtype=np.int64)
        for t in range(S):
            perm[t * 128:t * 128 + 64] = halves[0][t * 64:(t + 1) * 64]
            perm[t * 128 + 64:(t + 1) * 128] = halves[1][t * 64:(t + 1) * 64]
        perms.append(perm)

    # node -> (rank, core); pair index of node m: core*half + t*64 + (p%64)
    rank = np.zeros(n, dtype=np.int64)
    for c in range(nc_):
        m = perms[c] >= 0
        rank[perms[c][m]] = np.nonzero(m)[0]
    t_of = rank // 128
    p_of = rank % 128
    # q-major pair index: row = (p%64)*S + t, so each publish partition is
    # one contiguous 6272B DMA descriptor
    pair_of = (np.int64(cfg.half) * (np.arange(n) // nloc)
               + (p_of % 64) * S + t_of)
    zpair = cfg.half - 1       # block-relative filler pair (pad cell)

    # per-core per-lane streams; global per-slot column counts
    ca = np.zeros(S, dtype=np.int64)
    cb = np.zeros(S, dtype=np.int64)
    lane_lists = []
    for c in range(nc_):
        m = (dst // nloc) == c
        s_c, d_c = src[m], dst[m]
        r_c = rank[d_c]
        h_c = cls[np.arange(len(src))[m]]
        order = np.lexsort((pair_of[s_c], h_c, r_c))
        s_c, r_c, h_c = s_c[order], r_c[order], h_c[order]
        rows = pair_of[s_c]
        cnt0 = np.bincount(r_c[h_c == 0], minlength=cfg.npad)
        cnt1 = np.bincount(r_c[h_c == 1], minlength=cfg.npad)
        for t in range(S):
            ca[t] = max(ca[t], cnt0[t * 128:(t + 1) * 128].max())
            cb[t] = max(cb[t], cnt1[t * 128:(t + 1) * 128].max())
        lane_lists.append((rows[h_c == 0], r_c[h_c == 0], cnt0,
                           rows[h_c == 1], r_c[h_c == 1], cnt1))

    for t in range(S):
        assert ca[t] <= WCAP and cb[t] <= WCAP, (t, ca[t], cb[t])

    # slot-aligned windows per stream; idx columns are packed linearly in
    # stream order (A then B), so any chunking slices the same idx layout
    sbases = (0, int(ca.sum()))
    icol = int((ca.sum() + cb.sum()) * 8)

    def build_windows(cap, chunkcap, lead):
        res = ([], [])
        for sidx, cc in ((0, ca), (1, cb)):
            t = 0
            spos = 0
            while t < S:
                t0, cols = t, 0
                while t < S and cols + cc[t] <= cap and not (
                        t == S - 1 and t > t0):
                    cols += cc[t]
                    t += 1
                chunks = []
                done = 0
                while done < cols:
                    # small first chunk per stream: its descriptor
                    # generation is the DMA lead-in after the restride
                    mcols = min(lead if spos + done == 0 else chunkcap,
                                cols - done)
                    chunks.append(((sbases[sidx] + spos + done) * 8,
                                   int(mcols), int(done)))
                    done += mcols
                res[sidx].append((t0, t, int(cols), tuple(chunks)))
                spos += cols
        return (tuple(res[0]), tuple(res[1]))

    windows = build_windows(WCAP, MAXC, 24)

    windows3 = build_windows(WCAP3, 96, 48)

    # idx tensors: stream A flat columns then stream B, in slot order,
    # wrapped into 16 partitions and replicated x8 (SWDGE idx layout).
    idxs = np.zeros((nc_, 128, max(icol, 1)), dtype=np.int16)
    for c in range(nc_):
        rowsA, rA, cnt0, rowsB, rB, cnt1 = lane_lists[c]
        startsA = np.concatenate([[0], np.cumsum(cnt0)])
        startsB = np.concatenate([[0], np.cumsum(cnt1)])
        zfill = c * cfg.half + zpair
        flat_cols = []
        for cc, rows_, starts, cnt in ((ca, rowsA, startsA, cnt0),
                                       (cb, rowsB, startsB, cnt1)):
            for t in range(S):
                ranks = t * 128 + np.arange(128)
                for k in range(int(cc[t])):
                    col = np.full(128, zfill, dtype=np.int64)
                    have = cnt[ranks] > k
                    col[have] = rows_[starts[ranks[have]] + k]
                    flat_cols.append(col.astype(np.int16))
        if flat_cols:
            fc = np.stack(flat_cols)
            flat = fc.reshape(-1)
            wrapped = np.zeros((16, icol), np.int16)
            kk = np.arange(len(flat))
            wrapped[kk % 16, kk // 16] = flat
            idxs[c] = np.tile(wrapped, (8, 1))
    sched = Sched([int(x) for x in ca], [int(x) for x in cb],
                  windows, windows3, max(icol, 1))
    return deg, perms, sched, idxs


def build(cfg, sched):
    nc = bacc.Bacc("TRN2", target_bir_lowering=False, debug=False,
                   enable_asserts=False, num_devices=cfg.nc,
                   num_swdge_queues=4)
    S = cfg.slots
    NPF = S * 64
    NP = cfg.npad
    PR = cfg.prows
    HF = cfg.half
    ca, cb = sched.ca, sched.cb

    xl = nc.dram_tensor("xl", [128, NPF], F16, kind="ExternalInput").ap()
    xa = nc.dram_tensor("xa", [HF, D], F16, kind="ExternalInput").ap()
    xb = nc.dram_tensor("xb", [HF, D], F16, kind="ExternalInput").ap()
    degt = nc.dram_tensor("degt", [128, S], F32, kind="ExternalInput").ap()
    idx = nc.dram_tensor("idx", [128, sched.icols], I16,
                         kind="ExternalInput").ap()
    w1 = nc.dram_tensor("w1", [D, D], F32, kind="ExternalInput").ap()
    w2 = nc.dram_tensor("w2", [D, D], F32, kind="ExternalInput").ap()
    w3 = nc.dram_tensor("w3", [D, 1], F32, kind="ExternalInput").ap()
    gb = nc.dram_tensor("gb", [4, D], F32, kind="ExternalInput").ap()
    b3r = nc.dram_tensor("b3r", [128, 1], F32, kind="ExternalInput").ap()
    out = nc.dram_tensor("out", [1, NP], F32, kind="ExternalOutput").ap()

    inv_n = 1.0 / cfg.n

    with tile.TileContext(nc) as tc:
        with tc.tile_pool(name="const", bufs=1) as cpool, \
             tc.tile_pool(name="stga", bufs=3) as spool_a, \
             tc.tile_pool(name="stgb", bufs=3) as spool_b, \
             tc.tile_pool(name="ps", bufs=2, space="PSUM") as ppool, \
             tc.tile_pool(name="ps1", bufs=1, space="PSUM") as ppool1, \
             tc.tile_pool(name="dram", bufs=1, space="DRAM") as dpool:

            nc.gpsimd.load_library(library_config.mlp)

            # layer-1 AllGather inputs first in the DMA queue so AG_A can
            # launch ~15us earlier (everything else loads during it)
            agA = dpool.tile([HF, D], F16, name="agA")
            agB = dpool.tile([HF, D], F16, name="agB")
            nc.sync.dma_start(agA[:], xa[:])
            nc.sync.dma_start(agB[:], xb[:])

            ident = cpool.tile([128, 128], F32)
            make_identity(nc, ident[:])
            w1t = cpool.tile([D, D], F32)
            w2t = cpool.tile([D, D], F32)
            w3t = cpool.tile([D, 1], F32)
            gbt = cpool.tile([4, D], F32)
            b3rt = cpool.tile([128, 1], F32)
            epst = cpool.tile([D, 1], F32)
            nc.sync.dma_start(w1t[:], w1[:])
            nc.sync.dma_start(w2t[:], w2[:])
            nc.sync.dma_start(w3t[:], w3[:])
            nc.sync.dma_start(gbt[:], gb[:])
            nc.sync.dma_start(b3rt[:], b3r[:])
            nc.vector.memset(epst[:], EPS)
            idxt = cpool.tile([128, sched.icols], I16)
            nc.sync.dma_start(idxt[:], idx[:])
            degs = cpool.tile([128, S], F32)
            nc.sync.dma_start(degs[:], degt[:])

            # gbT [64, 4] = (g1, bt1, g2, bt2) columns
            pgb = ppool1.tile([D, 4], F32, name="pgb")
            nc.tensor.transpose(pgb[:], gbt[:], ident[:4, :4])
            gbs = cpool.tile([D, 4], F32)
            nc.vector.tensor_copy(out=gbs[:], in_=pgb[:])

            dinv = cpool.tile([128, S], F32)
            nc.scalar.sqrt(dinv[:], degs[:])
            nc.vector.reciprocal(dinv[:], dinv[:])
            hcast = cpool.tile([128, NPF], F16)
            wcast = cpool.tile([128, S], F16, name="wcast")
            wrow = cpool.tile([1, NP], F32, name="wrow")
            wq = cpool.tile([128, S], F32, name="wq")
            osum = cpool.tile([128, S], F32, name="osum")
            accA = cpool.tile([128, NPF], F16)
            acc_f32 = cpool.tile([128, NPF], F32, name="accf")
            nc.sync.dma_start(hcast[:], xl[:])      # host pre-scaled x*dinv
            dinv_exp = cpool.tile([128, NPF], F32, name="dinv_exp")
            for t in range(S):
                nc.vector.tensor_copy(
                    out=dinv_exp[:, t * 64:(t + 1) * 64],
                    in_=dinv[:, t:t + 1].to_broadcast([128, 64]))

            # wave publish buffers + AllGather outputs (pair-HBM shared)
            agC3 = dpool.tile([2 * HF, 1], F16, name="agC3")
            tabA = nc.dram_tensor("tabA", [PR, D], F16,
                                  addr_space="Shared").ap()
            tabB = nc.dram_tensor("tabB", [PR, D], F16,
                                  addr_space="Shared").ap()
            tabC3 = nc.dram_tensor("tabC3", [2 * PR, 1], F16,
                                   addr_space="Shared").ap()
            # paired gather table: row r = (wave-A node r, wave-B node r)
            tabs = [dpool.tile([PR, 2 * D], F16, name=f"tab{l}")
                    for l in range(3)]
            wrowd = dpool.tile([1, NP], F32, name="wrowd")
            ar_in = dpool.tile([D, 2], F32, name="ar_in")
            ar_out = [dpool.tile([NC * D, 2], F32, name=f"ar_out{l}")
                      for l in range(2)]

            yT = cpool.tile([D, NP], F32, name="yT")
            zT = cpool.tile([D, NP], F32, name="zT")
            st = cpool.tile([D, 2], F32, name="st")
            sta = cpool.tile([D, 16], F32, name="sta")
            stb = cpool.tile([D, 16], F32, name="stb")
            stw = cpool.tile([D, 16], F32, name="stw")
            stg = cpool.tile([D, 2], F32, name="stg")
            scb = cpool.tile([D, 4], F32, name="scb")
            msq = cpool.tile([D, 1], F32, name="msq")
            rstd = cpool.tile([D, 1], F32, name="rstd")

            nb = (NP + 511) // 512

            qi = 0
            for layer in range(3):
                # publish the two wave buffers from hcast (or, for the last
                # layer, scalar wcast = dinv*(h2@W3)) lane halves
                EW = 1 if layer == 2 else D     # table row payload elements
                srct = wcast if layer == 2 else hcast
                tab = tabs[layer]
                if layer < 2:
                    inA, inB = agA, agB
                    if layer != 0:    # layer 0 staged at kernel start
                        nc.sync.dma_start(
                            agA.rearrange("(q t) f -> q t f", t=S),
                            hcast[0:64, :].rearrange("q (t f) -> q t f", f=D))
                        nc.sync.dma_start(
                            agB.rearrange("(q t) f -> q t f", t=S),
                            hcast[64:128, :].rearrange("q (t f) -> q t f", f=D))
                    if cfg.nc > 1:
                        nc.gpsimd.collective_compute(
                            "AllGather", mybir.AluOpType.bypass,
                            replica_groups=[list(range(cfg.nc))],
                            ins=[inA.opt()], outs=[tabA.opt()],
                        )
                        nc.gpsimd.collective_compute(
                            "AllGather", mybir.AluOpType.bypass,
                            replica_groups=[list(range(cfg.nc))],
                            ins=[inB.opt()], outs=[tabB.opt()],
                        )
                    else:
                        nc.sync.dma_start(tabA[0:HF, :], inA[:])
                        nc.sync.dma_start(tabB[0:HF, :], inB[:])
                    # restride each wave into its half of the 256B paired
                    # rows; wave A's copy (and its gathers) overlap wave B's
                    # AllGather
                    nc.sync.dma_start(tab[:, 0:D], tabA[:])
                    nc.sync.dma_start(tab[:, D:2 * D], tabB[:])
                else:
                    # last layer: one tiny scalar AllGather for both waves
                    nc.sync.dma_start(
                        agC3[0:HF, :].rearrange("(q t) f -> q t f", t=S),
                        wcast[0:64, :].rearrange("q (t f) -> q t f", f=1))
                    nc.sync.dma_start(
                        agC3[HF:2 * HF, :].rearrange("(q t) f -> q t f", t=S),
                        wcast[64:128, :].rearrange("q (t f) -> q t f", f=1))
                    if cfg.nc > 1:
                        nc.gpsimd.collective_compute(
                            "AllGather", mybir.AluOpType.bypass,
                            replica_groups=[list(range(cfg.nc))],
                            ins=[agC3.opt()], outs=[tabC3.opt()],
                        )
                    else:
                        nc.sync.dma_start(tabC3[0:2 * HF, :], agC3[:])
                    for c in range(cfg.nc):
                        nc.sync.dma_start(
                            tab[c * HF:(c + 1) * HF, 0:1],
                            tabC3[c * 2 * HF:c * 2 * HF + HF, :])
                        nc.sync.dma_start(
                            tab[c * HF:(c + 1) * HF, D:D + 1],
                            tabC3[c * 2 * HF + HF:(c + 1) * 2 * HF, :])
                tviews = (tab[:, 0:EW], tab[:, D:D + EW])

                def emit_transpose(u):
                    pt = ppool.tile([D, 128], F32, tag="tp", name="pt")
                    nc.tensor.transpose(pt[:], acc_f32[:, u * 64:(u + 1) * 64],
                                        ident[:])
                    nc.vector.tensor_copy(out=yT[:, u * 128:(u + 1) * 128],
                                          in_=pt[:])

                def emit_zblock(b):
                    c0_, c1_ = b * 512, min(NP, b * 512 + 512)
                    pz = ppool.tile([D, 512], F32, tag="pz", name="pz")
                    nc.tensor.matmul(pz[:, :c1_ - c0_],
                                     (w1t, w2t)[layer][:],
                                     yT[:, c0_:c1_], start=True, stop=True)
                    nc.vector.tensor_copy(out=zT[:, c0_:c1_],
                                          in_=pz[:, :c1_ - c0_])
                    nc.scalar.activation(yT[:, c0_:c1_], zT[:, c0_:c1_],
                                         AF.Copy,
                                         accum_out=sta[:, b:b + 1])
                    nc.scalar.activation(yT[:, c0_:c1_], zT[:, c0_:c1_],
                                         AF.Square,
                                         accum_out=stb[:, b:b + 1])

                def fold(wt, base, k, ew):
                    """In-place binary-fold of k columns [128, k*ew] f16 at
                    column `base` of window tile wt; result lands at base."""
                    while k > 1:
                        h = k // 2      # top h columns fold onto the first h
                        nc.vector.tensor_add(
                            out=wt[:, base * ew:(base + h) * ew],
                            in0=wt[:, base * ew:(base + h) * ew],
                            in1=wt[:, (base + k - h) * ew:(base + k) * ew])
                        k -= h

                # ---- wave A: gathers + per-slot partial sums into accA;
                # ---- wave B: gathers + combine + drain ----
                wins = sched.windows3 if layer == 2 else sched.windows
                wcap_l = WCAP3 if layer == 2 else WCAP
                for sidx, cs, pool, target in (
                        (0, ca, spool_a, "A"), (1, cb, spool_b, "B")):
                    for (t0, t1, cols, chunks) in wins[sidx]:
                        wt = pool.tile([128, wcap_l * EW], F16,
                                       name=f"w{target}{EW}")
                        for (icol0, m, coff) in chunks:
                            nc.gpsimd.dma_gather(
                                wt[:, coff * EW:(coff + m) * EW]
                                .rearrange("p (m x) -> p m x", x=EW),
                                tviews[sidx], idxt[:, icol0:icol0 + 8 * m],
                                128 * m, 128 * m, EW, elem_step=2 * D,
                                single_packet=False, queue_num=qi % 4)
                            qi += 1
                        off = 0
                        for t in range(t0, t1):
                            k = cs[t]
                            if sidx == 0:
                                # fold slot; stash partial + self-loop term
                                if k > 0:
                                    fold(wt, off, k, EW)
                                    nc.vector.tensor_add(
                                        out=accA[:, t * EW:(t + 1) * EW],
                                        in0=wt[:, off * EW:(off + 1) * EW],
                                        in1=srct[:, t * EW:(t + 1) * EW])
                                else:
                                    nc.vector.tensor_copy(
                                        out=accA[:, t * EW:(t + 1) * EW],
                                        in_=srct[:, t * EW:(t + 1) * EW])
                            else:
                                # fold slot + A partial (has self-loop), drain
                                if k > 0:
                                    fold(wt, off, k, EW)
                                    nc.vector.tensor_add(
                                        out=wt[:, off * EW:(off + 1) * EW],
                                        in0=wt[:, off * EW:(off + 1) * EW],
                                        in1=accA[:, t * EW:(t + 1) * EW])
                                    srcap = wt[:, off * EW:(off + 1) * EW]
                                else:
                                    srcap = accA[:, t * EW:(t + 1) * EW]
                                if layer == 2:
                                    # out = dinv*agg, collected per slot
                                    nc.scalar.activation(
                                        osum[:, t:t + 1], srcap, AF.Copy,
                                        scale=dinv[:, t:t + 1])
                                else:
                                    nc.scalar.activation(
                                        acc_f32[:, t * 64:(t + 1) * 64],
                                        srcap, AF.Copy,
                                        scale=dinv[:, t:t + 1])
                                    if t >= 1:
                                        emit_transpose(t - 1)
                                        if (t - 1) % 4 == 3:
                                            emit_zblock((t - 1) // 4)
                            off += k

                if layer == 2:
                    nc.vector.tensor_add(
                        out=osum[:], in0=osum[:],
                        in1=b3rt[:, 0:1].to_broadcast([128, S]))
                    nc.sync.dma_start(
                        out.rearrange("a (t p) -> p (a t)", p=128), osum[:])
                else:
                    emit_transpose(S - 1)
                    for b in range((S - 2) // 4 + 1, nb):
                        emit_zblock(b)

                if layer < 2:
                    nc.scalar.activation(stw[:, 0:nb], sta[:, 0:nb],
                                         AF.Copy, scale=inv_n,
                                         accum_out=st[:, 0:1])
                    nc.scalar.activation(stw[:, 0:nb], stb[:, 0:nb],
                                         AF.Copy, scale=inv_n,
                                         accum_out=st[:, 1:2])
                    nc.sync.dma_start(ar_in[:], st[:])
                    if cfg.nc > 1:
                        nc.gpsimd.collective_compute(
                            "AllGather", mybir.AluOpType.bypass,
                            replica_groups=[list(range(cfg.nc))],
                            ins=[ar_in.opt()], outs=[ar_out[layer].opt()],
                        )
                        nc.sync.dma_start(
                            stw[:].rearrange("d (c s) -> d c s", s=2),
                            ar_out[layer].rearrange("(c d) s -> d c s", d=D))
                        nc.vector.tensor_add(out=stw[:, 0:8], in0=stw[:, 0:8],
                                             in1=stw[:, 8:16])
                        nc.vector.tensor_add(out=stw[:, 0:4], in0=stw[:, 0:4],
                                             in1=stw[:, 4:8])
                        nc.vector.tensor_add(out=stg[:], in0=stw[:, 0:2],
                                             in1=stw[:, 2:4])
                    else:
                        nc.sync.dma_start(ar_out[layer][0:D, :], ar_in[:])
                        nc.sync.dma_start(stg[:], ar_out[layer][0:D, :])
                    nc.vector.tensor_copy(out=scb[:, 0:2], in_=stg[:, 0:2])
                    nc.vector.tensor_mul(out=msq[:], in0=scb[:, 0:1],
                                         in1=scb[:, 0:1])
                    nc.vector.tensor_sub(out=scb[:, 1:2], in0=scb[:, 1:2],
                                         in1=msq[:])
                    nc.scalar.activation(rstd[:], scb[:, 1:2], AF.Sqrt,
                                         bias=epst[:, 0:1])
                    nc.vector.reciprocal(rstd[:], rstd[:])
                    nc.vector.tensor_mul(out=scb[:, 2:3],
                                         in0=gbs[:, 2 * layer:2 * layer + 1],
                                         in1=rstd[:])
                    nc.vector.tensor_mul(out=msq[:], in0=scb[:, 0:1],
                                         in1=scb[:, 2:3])
                    nc.vector.tensor_sub(out=scb[:, 3:4],
                                         in0=gbs[:, 2 * layer + 1:2 * layer + 2],
                                         in1=msq[:])
                    # h.T = Relu(scale*z + bias); hcast = dinv * h
                    for b_ in range(nb):
                        c0_, c1_ = b_ * 512, min(NP, b_ * 512 + 512)
                        nc.scalar.activation(yT[:, c0_:c1_], zT[:, c0_:c1_],
                                             AF.Relu, bias=scb[:, 3:4],
                                             scale=scb[:, 2:3])
                        if layer == 1:
                            # w row = W3^T @ h2 on the idle PE (Ahat commutes
                            # with the per-node map W3)
                            pw = ppool.tile([1, 512], F32, tag="pw",
                                            name="pw")
                            nc.tensor.matmul(pw[:, :c1_ - c0_], w3t[:],
                                             yT[:, c0_:c1_], start=True,
                                             stop=True)
                            # split psum->wrow copies across DVE/ACT so
                            # neither queue paces the rebuild alone
                            if b_ % 3 != 0:
                                nc.vector.tensor_copy(
                                    out=wrow[0:1, c0_:c1_],
                                    in_=pw[:, :c1_ - c0_])
                            else:
                                nc.scalar.activation(wrow[0:1, c0_:c1_],
                                                     pw[:, :c1_ - c0_],
                                                     AF.Copy)
                        ns_ = (c1_ - c0_) // 128
                        ph4 = ppool.tile([128, 4 * D], F32, tag="tp",
                                         name="ph")
                        for j, t in enumerate(range(c0_ // 128, c1_ // 128)):
                            nc.tensor.transpose(
                                ph4[:, j * D:(j + 1) * D],
                                yT[:, t * 128:(t + 1) * 128],
                                ident[:64, :64])
                        nc.vector.tensor_mul(
                            out=hcast[:, c0_ // 2:c1_ // 2],
                            in0=ph4[:, 0:ns_ * D],
                            in1=dinv_exp[:, c0_ // 2:c1_ // 2])
                    if layer == 1:
                        # bounce w row through DRAM into node-major [128, S]
                        nc.sync.dma_start(wrowd[:], wrow[:])
                        nc.sync.dma_start(
                            wq[:],
                            wrowd.rearrange("a (t p) -> p (a t)", p=128))
                        nc.vector.tensor_mul(out=wcast[:], in0=wq[:],
                                             in1=dinv[:])
                    # pad rows self-zero: their dinv is ~1e-19 (deg=1e38)

    nc.compile()
    return nc


def make_in_maps(cfg, inputs, deg, perms, idxs):
    x = np.asarray(inputs["x"], dtype=np.float32)
    in_maps = []
    for c in range(cfg.nc):
        pm = perms[c]
        m = pm >= 0
        xp = np.zeros((cfg.npad, D), np.float32)
        xp[m] = x[pm[m]] / np.sqrt(deg[pm[m]].astype(np.float32))[:, None]
        xlc = xp.reshape(cfg.slots, 128, D).transpose(1, 0, 2) \
                .reshape(128, -1).astype(np.float16)
        dg = np.full((cfg.npad,), 1e30, np.float32)
        dg[m] = deg[pm[m]].astype(np.float32)
        dgt = dg.reshape(cfg.slots, 128).T.copy()
        # wave publish buffers for layer 1: row q*S+t = xlc[q (+64), t*64:]
        x3 = xlc.reshape(128, cfg.slots, D)
        xa_ = x3[0:64].reshape(cfg.half, D)
        xb_ = x3[64:128].reshape(cfg.half, D)
        in_maps.append({
            "xl": np.ascontiguousarray(xlc),
            "xa": np.ascontiguousarray(xa_),
            "xb": np.ascontiguousarray(xb_),
            "degt": np.ascontiguousarray(dgt),
            "idx": np.ascontiguousarray(idxs[c]),
            "w1": np.asarray(inputs["W1"], np.float32),
            "w2": np.asarray(inputs["W2"], np.float32),
            "w3": np.asarray(inputs["W3"], np.float32).reshape(D, 1),
            "gb": np.stack([
                np.asarray(inputs["g1"], np.float32),
                np.asarray(inputs["bt1"], np.float32),
                np.asarray(inputs["g2"], np.float32),
                np.asarray(inputs["bt2"], np.float32)]),
            "b3r": np.full((128, 1), float(np.asarray(inputs["b3"])
                                           .reshape(-1)[0]), np.float32),
        })
    return in_maps


_CACHE = {}


def kernel(**inputs):
    cfg = Cfg(n_nodes=int(np.asarray(inputs["x"]).shape[0]), n_cores=NC)
    deg, perms, sched, idxs = host_prep(
        cfg, np.asarray(inputs["edge_index"]))

    key = (cfg.n, sched.key)
    if key not in _CACHE:
        _CACHE[key] = build(cfg, sched)
    nc = _CACHE[key]
    in_maps = make_in_maps(cfg, inputs, deg, perms, idxs)

    import concourse.bass_utils as bass_utils
    res = None
    for attempt in range(3):
        try:
            res = bass_utils.run_bass_kernel_spmd(
                nc, in_maps, core_ids=list(range(cfg.nc)))
            break
        except Exception:
            if attempt == 2:
                raise
    out = np.zeros((cfg.n,), np.float32)
    for c in range(cfg.nc):
        oc = np.asarray(res.results[c]["out"]).reshape(cfg.npad)
        m = perms[c] >= 0
        out[perms[c][m]] = oc[m]
    return out
